# revision 1
# baseline (speedup 1.0000x reference)
"""Trainium2 Bass kernel for nn_Contrastive_Loss (bs=8192, hidden=2048, 8 cores).

Math: reference(X, Y) = cl(X,Y) + cl(Y,X) where
  cl(A,B)[i] = -log(E_ii / (colsum_i(E) - E_ii)),  E = exp(norm(A) @ norm(B).T)
Since norm(Y)@norm(X).T = S.T, the second term's column sums are the first
term's row sums and the diagonals coincide.  With E = exp(S):
  out[i] = log(rowsum_i(E) - E_ii) + log(colsum_i(E) - E_ii) - 2*S_ii

Sharding: rows of X/Y split across 8 cores (1024 rows each).  Each core:
 - receives its raw X shard twice: row-major XS (for row norms + diagonal) and
   pre-transposed XT (hidden-major, the matmul stationary operand; the
   normalization scale 1/||x_i|| is folded into the exp activation's
   per-partition scale, so the matmul can consume raw X),
 - normalizes its Y shard to bf16, AllGathers normalized Y,
 - computes its 1024x8192 block of S via PE matmuls (K=2048), with exp+rowsum
   fused on ACT and column partial sums via ones-vector matmuls on PE,
 - ReduceScatter sums the column partials so each core gets exactly the 1024
   column sums matching its own rows.
"""

import numpy as np
from contextlib import ExitStack

import concourse.bass as bass
import concourse.bacc as bacc
import concourse.mybir as mybir
import concourse.tile as tile
from concourse.bass_utils import run_bass_kernel_spmd

FP32 = mybir.dt.float32
BF16 = mybir.dt.bfloat16

BS = 8192      # batch (rows of X and Y)
H = 2048       # hidden
NCORES = 8
RPC = BS // NCORES   # rows per core = 1024
EPS = 1e-8


def build(bs=BS, h=H, ncores=NCORES):
    rpc = bs // ncores
    mt_n = rpc // 128    # m-tiles per core
    ntw = min(512, bs)   # n-tile width (PSUM bank)
    nnt = bs // ntw      # n-tiles
    kt_n = h // 128      # k-tiles
    groups = [list(range(ncores))]

    nc = bacc.Bacc("TRN2", target_bir_lowering=False, num_devices=ncores)
    XS = nc.dram_tensor("XS", [rpc, h], FP32, kind="ExternalInput")
    XT = nc.dram_tensor("XT", [h, rpc], FP32, kind="ExternalInput")
    YS = nc.dram_tensor("YS", [rpc, h], FP32, kind="ExternalInput")
    OUT = nc.dram_tensor("OUT", [rpc, 1], FP32, kind="ExternalOutput")

    with tile.TileContext(nc) as tc, ExitStack() as ctx:
        dram = ctx.enter_context(tc.tile_pool(name="dram", bufs=1, space="DRAM"))
        prep = ctx.enter_context(tc.tile_pool(name="prep", bufs=3))
        junkp = ctx.enter_context(tc.tile_pool(name="junkp", bufs=2))
        small = ctx.enter_context(tc.tile_pool(name="small", bufs=4))
        stat = ctx.enter_context(tc.tile_pool(name="stat", bufs=1))
        wpool = ctx.enter_context(tc.tile_pool(name="wpool", bufs=1))
        ypool = ctx.enter_context(tc.tile_pool(name="ypool", bufs=3))
        epool = ctx.enter_context(tc.tile_pool(name="epool", bufs=18))
        cpool = ctx.enter_context(tc.tile_pool(name="cpool", bufs=2))
        psum = ctx.enter_context(tc.tile_pool(name="psum", bufs=4, space="PSUM"))
        cpsum = ctx.enter_context(tc.tile_pool(name="cpsum", bufs=2, space="PSUM"))

        YnD = dram.tile([rpc, h], BF16)
        # AllGather in 4 chunks so matmuls start before the full gather lands.
        # Chunk j gathers every rank's local Y rows [j*cw, (j+1)*cw).
        nch = 4 if mt_n % 4 == 0 else 1
        cw = rpc // nch                      # rows per chunk per rank
        YnC = [
            dram.tile([ncores * cw, h], BF16, addr_space="Shared",
                      name=f"YnC{j}", tag=f"YnC{j}")
            for j in range(nch)
        ]
        CS = dram.tile([bs], FP32)
        CSR = dram.tile([rpc], FP32)

        # persistent stats (p = row % 128, column m = row // 128)
        sdiag = stat.tile([128, mt_n], FP32)        # S_ii
        invx = stat.tile([128, mt_n], FP32)         # 1/||x_i||
        rstot = stat.tile([128, mt_n], FP32)        # rowsum(E)
        rsacc = stat.tile([128, mt_n * nnt], FP32)  # per-(m,nt) rowsums

        # ---- raw X^T -> SBUF as bf16 (cast in SWDGE dma) ----
        # xnt[p, k, m] = X[shard_row m, 128k+p]
        xnt = wpool.tile([128, kt_n, rpc], BF16)
        nc.gpsimd.dma_start(
            out=xnt[:], in_=XT.rearrange("(k p) m -> p k m", p=128)
        )

        # ---------------- Phase A: norms, normalized Y, diagonal ----------------
        def row_norm_inv(t, tag):
            """per-row 1/max(||row||, eps) for a [128, h] tile"""
            junk = junkp.tile([128, h], BF16, tag="junk", name="junk")
            ss = small.tile([128, 1], FP32, tag="ss", name="ss")
            nc.scalar.activation(
                junk[:], t[:], mybir.ActivationFunctionType.Square, accum_out=ss[:]
            )
            nrm = small.tile([128, 1], FP32, tag="nrm", name="nrm")
            nc.scalar.sqrt(nrm[:], ss[:])
            nc.vector.tensor_scalar_max(nrm[:], nrm[:], EPS)
            inv = small.tile([128, 1], FP32, tag="inv", name="inv")
            nc.vector.reciprocal(inv[:], nrm[:])
            return inv

        for m in range(mt_n):
            r0 = m * 128
            ys = prep.tile([128, h], FP32, tag="ldy", name="ldy")
            nc.gpsimd.dma_start(out=ys[:], in_=YS[r0 : r0 + 128, :])
            iy = row_norm_inv(ys, "y")
            yn = prep.tile([128, h], BF16, tag="yn", name="yn")
            nc.scalar.mul(yn[:], ys[:], iy[:])
            nc.gpsimd.dma_start(out=YnD[r0 : r0 + 128, :], in_=yn[:])

            xs = prep.tile([128, h], FP32, tag="ldx", name="ldx")
            nc.gpsimd.dma_start(out=xs[:], in_=XS[r0 : r0 + 128, :])
            ix = row_norm_inv(xs, "x")
            nc.vector.tensor_copy(invx[:, m : m + 1], ix[:])

            # diagonal: S_ii = (x_i . yn_i) / ||x_i||
            prod = junkp.tile([128, h], FP32, tag="prod", name="prod")
            nc.vector.tensor_mul(prod[:], xs[:], yn[:])
            sdr = small.tile([128, 1], FP32, tag="sdr", name="sdr")
            nc.vector.reduce_sum(sdr[:], prod[:], axis=mybir.AxisListType.X)
            nc.vector.tensor_mul(sdiag[:, m : m + 1], sdr[:], ix[:])

            # once this AG chunk's Y rows are staged, gather them
            if (m + 1) % (cw // 128) == 0:
                j = m // (cw // 128)
                nc.gpsimd.collective_compute(
                    "AllGather", mybir.AluOpType.bypass, replica_groups=groups,
                    ins=[YnD[j * cw : (j + 1) * cw, :]], outs=[YnC[j].opt()],
                )

        ones = stat.tile([128, 1], BF16)
        nc.vector.memset(ones[:], 1.0)

        # ---------------- Phase B: S block, exp, row/col sums ----------------
        # The ones-matmul column reduction of n-tile `nt` is interleaved into
        # n-tile `nt+1`'s S-matmul stream: by then every E tile of `nt` is
        # ready, so PE never stalls waiting on ACT's exp.
        def flush_colsum(prev_state, m):
            pnt, pcolps, pets = prev_state
            nc.tensor.matmul(
                pcolps[:], lhsT=ones[:], rhs=pets[m][:],
                start=(m == 0), stop=(m == mt_n - 1),
            )
            if m == mt_n - 1:
                cssb = cpool.tile([1, ntw], FP32, tag="cssb", name="cssb")
                nc.vector.tensor_copy(cssb[:], pcolps[:])
                nc.gpsimd.dma_start(
                    out=CS[pnt * ntw : (pnt + 1) * ntw], in_=cssb[:]
                )

        def tile_chunks(nt):
            """(chunk j, src row in YnC[j], nrows, dest offset) for n-tile nt"""
            if nch == 1:
                return [(0, nt * ntw, ntw, 0)]
            r = (nt * ntw) // rpc
            l0 = nt * ntw - r * rpc
            out = []
            for j in range(l0 // cw, (l0 + ntw - 1) // cw + 1):
                lo, hi = max(l0, j * cw), min(l0 + ntw, (j + 1) * cw)
                out.append((j, r * cw + lo - j * cw, hi - lo, lo - l0))
            return out

        # process n-tiles needing early AG chunks first
        nt_order = sorted(
            range(nnt), key=lambda nt: (max(c[0] for c in tile_chunks(nt)), nt)
        )
        prev = None
        for nt in nt_order:
            # ynt[p, k, n] = Yn[nt*ntw + n, 128k+p]  (xbar transpose DMAs)
            ynt = ypool.tile([128, kt_n, ntw], BF16, tag="ynt", name="ynt")
            for (j, srow, nrows, off) in tile_chunks(nt):
                nc.sync.dma_start(
                    out=ynt[:, :, off : off + nrows],
                    in_=YnC[j][srow : srow + nrows, :],
                    transpose=True,
                )
            colps = cpsum.tile([1, ntw], FP32, tag="colps", name="colps")
            ets = []
            for m in range(mt_n):
                ps = psum.tile([128, ntw], FP32, tag="S", name="S")
                for kt in range(kt_n):
                    nc.tensor.matmul(
                        ps[:],
                        lhsT=xnt[:, kt, m * 128 : (m + 1) * 128],
                        rhs=ynt[:, kt, :],
                        start=(kt == 0),
                        stop=(kt == kt_n - 1),
                    )
                et = epool.tile([128, ntw], BF16, tag="E", name="E")
                nc.scalar.activation(
                    et[:], ps[:], mybir.ActivationFunctionType.Exp,
                    scale=invx[:, m : m + 1],
                    accum_out=rsacc[:, m * nnt + nt : m * nnt + nt + 1],
                )
                ets.append(et)
                if prev is not None:
                    flush_colsum(prev, m)
            prev = (nt, colps, ets)
        for m in range(mt_n):
            flush_colsum(prev, m)

        # ---------------- ReduceScatter column sums ----------------
        nc.gpsimd.collective_compute(
            "ReduceScatter", mybir.AluOpType.add, replica_groups=groups,
            ins=[CS.opt()], outs=[CSR.opt()],
        )

        # ---------------- Finale ----------------
        csr = stat.tile([128, mt_n], FP32)
        nc.gpsimd.dma_start(out=csr[:], in_=CSR.rearrange("(a b) -> b a", b=128))
        for m in range(mt_n):
            nc.vector.reduce_sum(
                rstot[:, m : m + 1], rsacc[:, m * nnt : (m + 1) * nnt],
                axis=mybir.AxisListType.X,
            )
        edig = stat.tile([128, mt_n], FP32)
        nc.scalar.activation(edig[:], sdiag[:], mybir.ActivationFunctionType.Exp)
        negr = stat.tile([128, mt_n], FP32)
        nc.vector.tensor_sub(negr[:], rstot[:], edig[:])
        negc = stat.tile([128, mt_n], FP32)
        nc.vector.tensor_sub(negc[:], csr[:], edig[:])
        lr = stat.tile([128, mt_n], FP32)
        nc.scalar.activation(lr[:], negr[:], mybir.ActivationFunctionType.Ln)
        lcv = stat.tile([128, mt_n], FP32)
        nc.scalar.activation(lcv[:], negc[:], mybir.ActivationFunctionType.Ln)
        res = stat.tile([128, mt_n], FP32)
        nc.vector.tensor_add(res[:], lr[:], lcv[:])
        d2 = stat.tile([128, mt_n], FP32)
        nc.vector.tensor_scalar_mul(d2[:], sdiag[:], -2.0)
        nc.vector.tensor_add(res[:], res[:], d2[:])
        nc.gpsimd.dma_start(
            out=OUT.rearrange("(a b) c -> b (a c)", b=128), in_=res[:]
        )

    nc.compile()
    return nc


_CACHE = {}


def _get_nc():
    if "nc" not in _CACHE:
        _CACHE["nc"] = build()
    return _CACHE["nc"]


def make_in_maps(X, Y, ncores=NCORES, rpc=RPC):
    maps = []
    for i in range(ncores):
        xs = np.ascontiguousarray(X[i * rpc : (i + 1) * rpc])
        maps.append({
            "XS": xs,
            "XT": np.ascontiguousarray(xs.T),
            "YS": np.ascontiguousarray(Y[i * rpc : (i + 1) * rpc]),
        })
    return maps


def kernel(X, Y):
    X = np.ascontiguousarray(np.asarray(X, dtype=np.float32))
    Y = np.ascontiguousarray(np.asarray(Y, dtype=np.float32))
    assert X.shape == (BS, H) and Y.shape == (BS, H)
    nc = _get_nc()
    r = run_bass_kernel_spmd(nc, make_in_maps(X, Y), list(range(NCORES)))
    out = np.concatenate([r.results[i]["OUT"] for i in range(NCORES)], axis=0)
    return out.astype(np.float32)



# revision 2
# speedup vs baseline: 1.1623x; 1.1623x over previous
"""Trainium2 Bass kernel for nn_Contrastive_Loss (bs=8192, hidden=2048, 8 cores).

Math: reference(X, Y) = cl(X,Y) + cl(Y,X) where
  cl(A,B)[i] = -log(E_ii / (colsum_i(E) - E_ii)),  E = exp(norm(A) @ norm(B).T)
Since norm(Y)@norm(X).T = S.T, the second term's column sums are the first
term's row sums and the diagonals coincide.  With E = exp(S):
  out[i] = log(rowsum_i(E) - E_ii) + log(colsum_i(E) - E_ii) - 2*S_ii

v2 sharding: rows of X split across 8 cores (1024 rows each); the full Y^T
(host-transposed, bf16) is replicated to every core, eliminating the large
normalized-Y AllGather of v1 (the dominant cost in the timeline sim).  Each
core:
 - computes row norms of its X and Y shards plus the X.Y diagonal dot,
 - AllGathers the tiny inv||y|| vector (4KB) so every core can column-scale,
 - streams Y^T n-tiles from DRAM, scales them by inv||y|| (broadcast via a
   K=1 PE matmul into PSUM, then in-place DVE multiplies, software-pipelined
   one n-tile ahead so PE never waits),
 - computes its 1024x8192 block of S via PE matmuls (K=2048) with the
   1/||x_i|| row scale folded into the exp activation, rowsums via ACT accum,
   column partial sums via ones-vector matmuls interleaved one n-tile behind,
 - ReduceScatters the column sums (32KB) so each core gets the 1024 column
   sums matching its own rows.
"""

import numpy as np
from contextlib import ExitStack

import concourse.bass as bass
import concourse.bacc as bacc
import concourse.mybir as mybir
import concourse.tile as tile
from concourse.bass_utils import run_bass_kernel_spmd

FP32 = mybir.dt.float32
BF16 = mybir.dt.bfloat16

BS = 8192      # batch (rows of X and Y)
H = 2048       # hidden
NCORES = 8
RPC = BS // NCORES   # rows per core = 1024
EPS = 1e-8


def build(bs=BS, h=H, ncores=NCORES):
    rpc = bs // ncores
    mt_n = rpc // 128    # m-tiles per core = 8
    ntw = min(512, bs)   # n-tile width (PSUM bank)
    nnt = bs // ntw      # n-tiles = 16
    kt_n = h // 128      # k-tiles = 16
    groups = [list(range(ncores))]

    nc = bacc.Bacc("TRN2", target_bir_lowering=False, num_devices=ncores)
    XS = nc.dram_tensor("XS", [rpc, h], FP32, kind="ExternalInput")
    XT = nc.dram_tensor("XT", [h, rpc], FP32, kind="ExternalInput")
    YS = nc.dram_tensor("YS", [rpc, h], FP32, kind="ExternalInput")
    YT = nc.dram_tensor("YT", [h, bs], BF16, kind="ExternalInput")
    OUT = nc.dram_tensor("OUT", [rpc, 1], FP32, kind="ExternalOutput")

    with tile.TileContext(nc) as tc, ExitStack() as ctx:
        dram = ctx.enter_context(tc.tile_pool(name="dram", bufs=1, space="DRAM"))
        prep = ctx.enter_context(tc.tile_pool(name="prep", bufs=3))
        junkp = ctx.enter_context(tc.tile_pool(name="junkp", bufs=2))
        small = ctx.enter_context(tc.tile_pool(name="small", bufs=4))
        stat = ctx.enter_context(tc.tile_pool(name="stat", bufs=1))
        wpool = ctx.enter_context(tc.tile_pool(name="wpool", bufs=1))
        ypool = ctx.enter_context(tc.tile_pool(name="ypool", bufs=3))
        epool = ctx.enter_context(tc.tile_pool(name="epool", bufs=18))
        cpool = ctx.enter_context(tc.tile_pool(name="cpool", bufs=2))
        rowp = ctx.enter_context(tc.tile_pool(name="rowp", bufs=2))
        psum = ctx.enter_context(tc.tile_pool(name="psum", bufs=4, space="PSUM"))
        cpsum = ctx.enter_context(tc.tile_pool(name="cpsum", bufs=2, space="PSUM"))
        spsum = ctx.enter_context(tc.tile_pool(name="spsum", bufs=2, space="PSUM"))

        INVYL = dram.tile([rpc], FP32)
        INVY = dram.tile([bs], FP32, addr_space="Shared", name="INVY", tag="INVY")
        CS = dram.tile([bs], FP32)
        CSR = dram.tile([rpc], FP32)

        # persistent stats (p = row % 128, column m = row // 128)
        invx = stat.tile([128, mt_n], FP32)         # 1/||x_i||
        invy_l = stat.tile([128, mt_n], FP32)       # 1/||y_i|| (local shard)
        xy = stat.tile([128, mt_n], FP32)           # x_i . y_i (raw)
        rstot = stat.tile([128, mt_n], FP32)        # rowsum(E)
        rsacc = stat.tile([128, mt_n * nnt], FP32)  # per-(m,nt) rowsums

        # ---- raw X^T -> SBUF as bf16 (cast in SWDGE dma) ----
        # xnt[p, k, m] = X[shard_row m, 128k+p]
        xnt = wpool.tile([128, kt_n, rpc], BF16)
        nc.gpsimd.dma_start(
            out=xnt[:], in_=XT.rearrange("(k p) m -> p k m", p=128)
        )

        # ---------------- Phase A: row norms + diagonal dot ----------------
        def row_norm_inv(t):
            """per-row 1/max(||row||, eps) for a [128, h] tile"""
            junk = junkp.tile([128, h], BF16, tag="junk", name="junk")
            ss = small.tile([128, 1], FP32, tag="ss", name="ss")
            nc.scalar.activation(
                junk[:], t[:], mybir.ActivationFunctionType.Square, accum_out=ss[:]
            )
            nrm = small.tile([128, 1], FP32, tag="nrm", name="nrm")
            nc.scalar.sqrt(nrm[:], ss[:])
            nc.vector.tensor_scalar_max(nrm[:], nrm[:], EPS)
            inv = small.tile([128, 1], FP32, tag="inv", name="inv")
            nc.vector.reciprocal(inv[:], nrm[:])
            return inv

        for m in range(mt_n):
            r0 = m * 128
            ys = prep.tile([128, h], FP32, tag="ldy", name="ldy")
            nc.gpsimd.dma_start(out=ys[:], in_=YS[r0 : r0 + 128, :])
            iy = row_norm_inv(ys)
            nc.vector.tensor_copy(invy_l[:, m : m + 1], iy[:])

            xs = prep.tile([128, h], FP32, tag="ldx", name="ldx")
            nc.gpsimd.dma_start(out=xs[:], in_=XS[r0 : r0 + 128, :])
            ix = row_norm_inv(xs)
            nc.vector.tensor_copy(invx[:, m : m + 1], ix[:])

            # raw diagonal dot: x_i . y_i (normalization applied in finale)
            prod = junkp.tile([128, h], FP32, tag="prod", name="prod")
            nc.vector.tensor_mul(prod[:], xs[:], ys[:])
            nc.vector.reduce_sum(xy[:, m : m + 1], prod[:], axis=mybir.AxisListType.X)

        # inv||y|| shard -> DRAM row-major -> AllGather (4KB)
        nc.gpsimd.dma_start(
            out=INVYL.rearrange("(a b) -> b a", b=128), in_=invy_l[:]
        )
        nc.gpsimd.collective_compute(
            "AllGather", mybir.AluOpType.bypass, replica_groups=groups,
            ins=[INVYL.opt()], outs=[INVY.opt()],
        )

        ones = stat.tile([128, 1], BF16)
        nc.vector.memset(ones[:], 1.0)
        ones1 = stat.tile([1, 128], FP32)
        nc.vector.memset(ones1[:], 1.0)

        # ---------------- Phase B: S block, exp, row/col sums ----------------
        # The ones-matmul column reduction of n-tile `nt` is interleaved into
        # n-tile `nt+1`'s S-matmul stream: by then every E tile of `nt` is
        # ready, so PE never stalls waiting on ACT's exp.
        def flush_colsum(prev_state, m):
            pnt, pcolps, pets = prev_state
            nc.tensor.matmul(
                pcolps[:], lhsT=ones[:], rhs=pets[m][:],
                start=(m == 0), stop=(m == mt_n - 1),
            )
            if m == mt_n - 1:
                cssb = cpool.tile([1, ntw], FP32, tag="cssb", name="cssb")
                nc.vector.tensor_copy(cssb[:], pcolps[:])
                nc.gpsimd.dma_start(
                    out=CS[pnt * ntw : (pnt + 1) * ntw], in_=cssb[:]
                )

        def load_ynt(nt):
            """DMA the raw Y^T n-tile + the inv||y|| row it needs."""
            ynt = ypool.tile([128, kt_n, ntw], BF16, tag="ynt", name="ynt")
            nc.sync.dma_start(
                out=ynt[:],
                in_=YT[:, nt * ntw : (nt + 1) * ntw].rearrange(
                    "(k p) n -> p k n", p=128
                ),
            )
            sclrow = rowp.tile([1, ntw], FP32, tag="sclrow", name="sclrow")
            nc.sync.dma_start(
                out=sclrow[:], in_=INVY[nt * ntw : (nt + 1) * ntw]
            )
            return ynt, sclrow

        def bcast_scl(sclrow):
            """inv||y|| row -> [128, ntw] PSUM tile via K=1 matmul broadcast"""
            scl = spsum.tile([128, ntw], FP32, tag="scl", name="scl")
            nc.tensor.matmul(scl[:], lhsT=ones1[:], rhs=sclrow[:])
            return scl

        def scale_ynt(ynt, scl, kt):
            nc.vector.tensor_mul(ynt[:, kt, :], ynt[:, kt, :], scl[:])

        pend = load_ynt(0)
        pend_scl = bcast_scl(pend[1])
        for kt in range(kt_n):
            scale_ynt(pend[0], pend_scl, kt)

        prev = None
        for nt in range(nnt):
            ynt = pend[0]
            if nt + 1 < nnt:
                nxt = load_ynt(nt + 1)
            colps = cpsum.tile([1, ntw], FP32, tag="colps", name="colps")
            ets = []
            nxt_scl = None
            for m in range(mt_n):
                ps = psum.tile([128, ntw], FP32, tag="S", name="S")
                for kt in range(kt_n):
                    nc.tensor.matmul(
                        ps[:],
                        lhsT=xnt[:, kt, m * 128 : (m + 1) * 128],
                        rhs=ynt[:, kt, :],
                        start=(kt == 0),
                        stop=(kt == kt_n - 1),
                    )
                # prep next n-tile's scale while this one's matmuls stream
                if nt + 1 < nnt:
                    if m == 0:
                        nxt_scl = bcast_scl(nxt[1])
                    elif m - 1 < kt_n and m >= 1:
                        for kt in range(2 * (m - 1), min(2 * m, kt_n)):
                            scale_ynt(nxt[0], nxt_scl, kt)
                et = epool.tile([128, ntw], BF16, tag="E", name="E")
                nc.scalar.activation(
                    et[:], ps[:], mybir.ActivationFunctionType.Exp,
                    scale=invx[:, m : m + 1],
                    accum_out=rsacc[:, m * nnt + nt : m * nnt + nt + 1],
                )
                ets.append(et)
                if prev is not None:
                    flush_colsum(prev, m)
            if nt + 1 < nnt:
                for kt in range(2 * (mt_n - 1), kt_n):
                    scale_ynt(nxt[0], nxt_scl, kt)
                pend = nxt
            prev = (nt, colps, ets)
        for m in range(mt_n):
            flush_colsum(prev, m)

        # ---------------- ReduceScatter column sums ----------------
        nc.gpsimd.collective_compute(
            "ReduceScatter", mybir.AluOpType.add, replica_groups=groups,
            ins=[CS.opt()], outs=[CSR.opt()],
        )

        # ---------------- Finale ----------------
        csr = stat.tile([128, mt_n], FP32)
        nc.gpsimd.dma_start(out=csr[:], in_=CSR.rearrange("(a b) -> b a", b=128))
        for m in range(mt_n):
            nc.vector.reduce_sum(
                rstot[:, m : m + 1], rsacc[:, m * nnt : (m + 1) * nnt],
                axis=mybir.AxisListType.X,
            )
        sdiag = stat.tile([128, mt_n], FP32)
        nc.vector.tensor_mul(sdiag[:], xy[:], invx[:])
        nc.vector.tensor_mul(sdiag[:], sdiag[:], invy_l[:])
        edig = stat.tile([128, mt_n], FP32)
        nc.scalar.activation(edig[:], sdiag[:], mybir.ActivationFunctionType.Exp)
        negr = stat.tile([128, mt_n], FP32)
        nc.vector.tensor_sub(negr[:], rstot[:], edig[:])
        negc = stat.tile([128, mt_n], FP32)
        nc.vector.tensor_sub(negc[:], csr[:], edig[:])
        lr = stat.tile([128, mt_n], FP32)
        nc.scalar.activation(lr[:], negr[:], mybir.ActivationFunctionType.Ln)
        lcv = stat.tile([128, mt_n], FP32)
        nc.scalar.activation(lcv[:], negc[:], mybir.ActivationFunctionType.Ln)
        res = stat.tile([128, mt_n], FP32)
        nc.vector.tensor_add(res[:], lr[:], lcv[:])
        d2 = stat.tile([128, mt_n], FP32)
        nc.vector.tensor_scalar_mul(d2[:], sdiag[:], -2.0)
        nc.vector.tensor_add(res[:], res[:], d2[:])
        nc.gpsimd.dma_start(
            out=OUT.rearrange("(a b) c -> b (a c)", b=128), in_=res[:]
        )

    nc.compile()
    return nc


_CACHE = {}


def _get_nc():
    if "nc" not in _CACHE:
        _CACHE["nc"] = build()
    return _CACHE["nc"]


def make_in_maps(X, Y, ncores=NCORES, rpc=RPC):
    import ml_dtypes

    yt = np.ascontiguousarray(Y.T).astype(ml_dtypes.bfloat16)
    maps = []
    for i in range(ncores):
        xs = np.ascontiguousarray(X[i * rpc : (i + 1) * rpc])
        maps.append({
            "XS": xs,
            "XT": np.ascontiguousarray(xs.T),
            "YS": np.ascontiguousarray(Y[i * rpc : (i + 1) * rpc]),
            "YT": yt,
        })
    return maps


def kernel(X, Y):
    X = np.ascontiguousarray(np.asarray(X, dtype=np.float32))
    Y = np.ascontiguousarray(np.asarray(Y, dtype=np.float32))
    assert X.shape == (BS, H) and Y.shape == (BS, H)
    nc = _get_nc()
    r = run_bass_kernel_spmd(nc, make_in_maps(X, Y), list(range(NCORES)))
    out = np.concatenate([r.results[i]["OUT"] for i in range(NCORES)], axis=0)
    return out.astype(np.float32)


# revision 3
# speedup vs baseline: 1.1654x; 1.0026x over previous
"""Trainium2 Bass kernel for nn_Contrastive_Loss (bs=8192, hidden=2048, 8 cores).

Math: reference(X, Y) = cl(X,Y) + cl(Y,X); with E = exp(S), S = cosine sims:
  out[i] = log(rowsum_i(E) - E_ii) + log(colsum_i(E) - E_ii) - 2*S_ii

v4: zero collectives (first collective costs ~1.5ms pipelined here) AND a
minimal instruction count (per-call cost in this deployment scales ~0.35us
per NEFF instruction).  Each core gets full X^T/Y^T in fp8 plus row-major
bf16 copies for norms, and computes both its row block E[R_c, :]
(-> rowsums) and transposed column block E[:, R_c]^T (-> colsums via ACT
free-axis accumulation).  fp8 DoubleRow matmuls contract K=256 per
instruction (2x PE, half the matmul instructions); norms are one ACT
Square+accum instruction per 128 rows; E tiles are never materialized.
"""

import numpy as np
from contextlib import ExitStack

import concourse.bass as bass
import concourse.bacc as bacc
import concourse.mybir as mybir
import concourse.tile as tile
from concourse.bass_utils import run_bass_kernel_spmd

FP32 = mybir.dt.float32
BF16 = mybir.dt.bfloat16
FP8 = mybir.dt.float8e4
DR = mybir.MatmulPerfMode.DoubleRow

BS = 8192
H = 2048
NCORES = 8
RPC = BS // NCORES
EPS = 1e-8


def build(bs=BS, h=H, ncores=NCORES):
    rpc = bs // ncores
    mt_n = rpc // 128    # local 128-row blocks = 8
    ntw = 512            # moving n-tile width (PSUM bank)
    nnt = bs // ntw      # n-tiles over the full batch = 16
    kp_n = h // 256      # DoubleRow k-pair tiles = 8
    nb = bs // 128       # 128-row norm blocks over full batch = 64

    nc = bacc.Bacc("TRN2", target_bir_lowering=False, num_devices=ncores)
    XTF = nc.dram_tensor("XTF", [h, bs], FP8, kind="ExternalInput")
    YTF = nc.dram_tensor("YTF", [h, bs], FP8, kind="ExternalInput")
    XTL = nc.dram_tensor("XTL", [h, rpc], FP8, kind="ExternalInput")
    YTL = nc.dram_tensor("YTL", [h, rpc], FP8, kind="ExternalInput")
    XSF = nc.dram_tensor("XSF", [bs, h], BF16, kind="ExternalInput")
    YSF = nc.dram_tensor("YSF", [bs, h], BF16, kind="ExternalInput")
    XSL = nc.dram_tensor("XSL", [rpc, h], BF16, kind="ExternalInput")
    YSL = nc.dram_tensor("YSL", [rpc, h], BF16, kind="ExternalInput")
    OUT = nc.dram_tensor("OUT", [rpc, 1], FP32, kind="ExternalOutput")

    with tile.TileContext(nc) as tc, ExitStack() as ctx:
        dram = ctx.enter_context(tc.tile_pool(name="dram", bufs=1, space="DRAM"))
        stat = ctx.enter_context(tc.tile_pool(name="stat", bufs=1))
        wpool = ctx.enter_context(tc.tile_pool(name="wpool", bufs=1))
        ypool = ctx.enter_context(tc.tile_pool(name="ypool", bufs=2))
        prep = ctx.enter_context(tc.tile_pool(name="prep", bufs=2))
        scp = ctx.enter_context(tc.tile_pool(name="scp", bufs=2))
        junkp = ctx.enter_context(tc.tile_pool(name="junkp", bufs=2))
        pjp = ctx.enter_context(tc.tile_pool(name="pjp", bufs=1))
        sclp = ctx.enter_context(tc.tile_pool(name="sclp", bufs=2))
        rowp = ctx.enter_context(tc.tile_pool(name="rowp", bufs=1))
        psS = ctx.enter_context(tc.tile_pool(name="psS", bufs=4, space="PSUM"))
        psB = ctx.enter_context(tc.tile_pool(name="psB", bufs=2, space="PSUM"))

        INVX = dram.tile([bs], FP32)
        INVY = dram.tile([bs], FP32)

        # stats; global row g at [g % 128, g // 128], local row r likewise
        xnorm2 = stat.tile([128, nb], FP32)
        ynorm2 = stat.tile([128, nb], FP32)
        invx_a = stat.tile([128, nb], FP32)
        invy_a = stat.tile([128, nb], FP32)
        ln2x = stat.tile([128, mt_n], FP32)
        ln2y = stat.tile([128, mt_n], FP32)
        invx_l = stat.tile([128, mt_n], FP32)
        invy_l = stat.tile([128, mt_n], FP32)
        xy = stat.tile([128, mt_n], FP32)
        rsacc = stat.tile([128, mt_n * nnt], FP32)
        csacc = stat.tile([128, mt_n * nnt], FP32)

        ones1 = stat.tile([1, 128], FP32)
        nc.vector.memset(ones1[:], 1.0)

        # full X^T resident in SBUF (fp8, 16MB), loaded in n-tile chunks so
        # pass C can start before the tail arrives
        xnt = wpool.tile([128, kp_n, 2, bs], FP8)
        for nt in range(nnt):
            nc.gpsimd.dma_start(
                out=xnt[:, :, :, nt * ntw : (nt + 1) * ntw],
                in_=XTF[:, nt * ntw : (nt + 1) * ntw].rearrange(
                    "(k i p) m -> p k i m", p=128, i=2
                ),
            )
        xntl = stat.tile([128, kp_n, 2, rpc], FP8)
        nc.gpsimd.dma_start(
            out=xntl[:], in_=XTL.rearrange("(k i p) m -> p k i m", p=128, i=2)
        )
        yntl = stat.tile([128, kp_n, 2, rpc], FP8)
        nc.gpsimd.dma_start(
            out=yntl[:], in_=YTL.rearrange("(k i p) m -> p k i m", p=128, i=2)
        )

        def norm_sq(src, b, out_slot):
            t = prep.tile([128, h], BF16, tag="nt", name="nt")
            nc.sync.dma_start(out=t[:], in_=src[b * 128 : (b + 1) * 128, :])
            junk = junkp.tile([128, h], BF16, tag="nj", name="nj", bufs=1)
            nc.scalar.activation(
                junk[:], t[:], mybir.ActivationFunctionType.Square,
                accum_out=out_slot[:, b : b + 1],
            )
            return t

        def finish_inv(n2, inv):
            nc.scalar.sqrt(inv[:], n2[:])
            nc.vector.tensor_scalar_max(inv[:], inv[:], EPS)
            nc.vector.reciprocal(inv[:], inv[:])

        # ---- local stats (diag needs both local tiles live) ----
        for m in range(mt_n):
            xt = norm_sq(XSL, m, ln2x)
            yt_ = prep.tile([128, h], BF16, tag="nt2", name="nt2", bufs=1)
            nc.sync.dma_start(out=yt_[:], in_=YSL[m * 128 : (m + 1) * 128, :])
            junk = junkp.tile([128, h], BF16, tag="nj", name="nj", bufs=1)
            nc.scalar.activation(
                junk[:], yt_[:], mybir.ActivationFunctionType.Square,
                accum_out=ln2y[:, m : m + 1],
            )
            prod = pjp.tile([128, h], BF16, tag="pj", name="pj")
            nc.vector.tensor_mul(prod[:], xt[:], yt_[:])
            nc.vector.reduce_sum(
                xy[:, m : m + 1], prod[:], axis=mybir.AxisListType.X
            )
        finish_inv(ln2x, invx_l)
        finish_inv(ln2y, invy_l)

        # ---- full X/Y norms (one ACT Square per 128 rows) ----
        for b in range(nb):
            norm_sq(XSF, b, xnorm2)
        finish_inv(xnorm2, invx_a)
        nc.gpsimd.dma_start(
            out=INVX.rearrange("(a b) -> b a", b=128), in_=invx_a[:]
        )
        for b in range(nb):
            norm_sq(YSF, b, ynorm2)
        finish_inv(ynorm2, invy_a)
        nc.gpsimd.dma_start(
            out=INVY.rearrange("(a b) -> b a", b=128), in_=invy_a[:]
        )

        def load_scl(src, nt):
            row = rowp.tile([1, ntw], FP32, tag="row", name="row")
            nc.sync.dma_start(out=row[:], in_=src[nt * ntw : (nt + 1) * ntw])
            bp = psB.tile([128, ntw], FP32, tag="bp", name="bp")
            nc.tensor.matmul(bp[:], lhsT=ones1[:], rhs=row[:])
            scl = sclp.tile([128, ntw], BF16, tag="scl", name="scl")
            nc.vector.tensor_copy(scl[:], bp[:])
            return scl

        def pass_block(stationary, scl, inv_p, acc, m, nt, moving_sl):
            ps = psS.tile([128, ntw], FP32, tag="S", name="S")
            for t in range(kp_n):
                nc.tensor.matmul(
                    ps[:],
                    lhsT=stationary[:, t, :, m * 128 : (m + 1) * 128],
                    rhs=moving_sl(t),
                    start=(t == 0), stop=(t == kp_n - 1), perf_mode=DR,
                )
            sc = scp.tile([128, ntw], BF16, tag="sc", name="sc")
            nc.vector.tensor_mul(sc[:], ps[:], scl[:])
            et = junkp.tile([128, ntw], BF16, tag="et", name="et")
            nc.scalar.activation(
                et[:], sc[:], mybir.ActivationFunctionType.Exp,
                scale=inv_p[:, m : m + 1],
                accum_out=acc[:, m * nnt + nt : m * nnt + nt + 1],
            )

        # ---- pass C (colsums): local Y^T stationary x resident X^T ----
        for nt in range(nnt):
            scl = load_scl(INVX, nt)
            for m in range(mt_n):
                pass_block(
                    yntl, scl, invy_l, csacc, m, nt,
                    lambda t: xnt[:, t, :, nt * ntw : (nt + 1) * ntw],
                )

        # ---- pass R (rowsums): local X^T stationary x streamed Y^T ----
        for nt in range(nnt):
            scl = load_scl(INVY, nt)
            ynt = ypool.tile([128, kp_n, 2, ntw], FP8, tag="ynt", name="ynt")
            nc.gpsimd.dma_start(
                out=ynt[:],
                in_=YTF[:, nt * ntw : (nt + 1) * ntw].rearrange(
                    "(k i p) n -> p k i n", p=128, i=2
                ),
            )
            for m in range(mt_n):
                pass_block(
                    xntl, scl, invx_l, rsacc, m, nt, lambda t: ynt[:, t, :, :]
                )

        # ---- finale ----
        rstot = stat.tile([128, mt_n], FP32)
        cstot = stat.tile([128, mt_n], FP32)
        for m in range(mt_n):
            nc.vector.reduce_sum(
                rstot[:, m : m + 1], rsacc[:, m * nnt : (m + 1) * nnt],
                axis=mybir.AxisListType.X,
            )
            nc.vector.reduce_sum(
                cstot[:, m : m + 1], csacc[:, m * nnt : (m + 1) * nnt],
                axis=mybir.AxisListType.X,
            )
        sdiag = stat.tile([128, mt_n], FP32)
        nc.vector.tensor_mul(sdiag[:], xy[:], invx_l[:])
        nc.vector.tensor_mul(sdiag[:], sdiag[:], invy_l[:])
        edig = stat.tile([128, mt_n], FP32)
        nc.scalar.activation(edig[:], sdiag[:], mybir.ActivationFunctionType.Exp)
        negr = stat.tile([128, mt_n], FP32)
        nc.vector.tensor_sub(negr[:], rstot[:], edig[:])
        negc = stat.tile([128, mt_n], FP32)
        nc.vector.tensor_sub(negc[:], cstot[:], edig[:])
        lr = stat.tile([128, mt_n], FP32)
        nc.scalar.activation(lr[:], negr[:], mybir.ActivationFunctionType.Ln)
        lcv = stat.tile([128, mt_n], FP32)
        nc.scalar.activation(lcv[:], negc[:], mybir.ActivationFunctionType.Ln)
        res = stat.tile([128, mt_n], FP32)
        nc.vector.tensor_add(res[:], lr[:], lcv[:])
        d2 = stat.tile([128, mt_n], FP32)
        nc.vector.tensor_scalar_mul(d2[:], sdiag[:], -2.0)
        nc.vector.tensor_add(res[:], res[:], d2[:])
        nc.gpsimd.dma_start(
            out=OUT.rearrange("(a b) c -> b (a c)", b=128), in_=res[:]
        )

    nc.compile()
    return nc


_CACHE = {}


def _get_nc():
    if "nc" not in _CACHE:
        _CACHE["nc"] = build()
    return _CACHE["nc"]


def make_in_maps(X, Y, ncores=NCORES, rpc=RPC):
    import ml_dtypes

    f8 = ml_dtypes.float8_e4m3
    b16 = ml_dtypes.bfloat16
    xtf = np.ascontiguousarray(X.T).astype(f8)
    ytf = np.ascontiguousarray(Y.T).astype(f8)
    xsf = X.astype(b16)
    ysf = Y.astype(b16)
    maps = []
    for i in range(ncores):
        sl = slice(i * rpc, (i + 1) * rpc)
        maps.append({
            "XTF": xtf,
            "YTF": ytf,
            "XTL": np.ascontiguousarray(xtf[:, sl]),
            "YTL": np.ascontiguousarray(ytf[:, sl]),
            "XSF": xsf,
            "YSF": ysf,
            "XSL": np.ascontiguousarray(xsf[sl]),
            "YSL": np.ascontiguousarray(ysf[sl]),
        })
    return maps


def kernel(X, Y):
    X = np.ascontiguousarray(np.asarray(X, dtype=np.float32))
    Y = np.ascontiguousarray(np.asarray(Y, dtype=np.float32))
    assert X.shape == (BS, H) and Y.shape == (BS, H)
    nc = _get_nc()
    r = run_bass_kernel_spmd(nc, make_in_maps(X, Y), list(range(NCORES)))
    out = np.concatenate([r.results[i]["OUT"] for i in range(NCORES)], axis=0)
    return out.astype(np.float32)
